# revision 1
# baseline (speedup 1.0000x reference)
"""COMET MoE-routing kernel for one TRN2 chip (8 NeuronCores).

Strategy: pure data parallelism over the batch (8192 -> 8 x 1024). Per core:
  - routing matmul  R^T = [sel_w; leaf_w] @ x^T  on the TensorEngine (fp32,
    contraction over d_in in 8 chunks of 128, inputs PE-transposed on chip)
  - smooth-step gates, binary-tree path products, masked softmax-weight
    computation on VectorE/ScalarE in (batch-partition, slot-free) layout
  - y = sum_e h[:, :, e] * w_norm[:, e] as a chain of fused
    scalar_tensor_tensor multiply-accumulates against the streamed h tiles
  - per-sample outputs y (B,768), w_norm (B,16), entropy (B,1); the tiny
    cross-batch means / thresholds / reg scalar are finished on the host.
No collectives are needed: every cross-batch reduction is 16 floats per core.
"""
from contextlib import ExitStack

import numpy as np

import concourse.bass as bass
import concourse.tile as tile
from concourse import bacc, masks, mybir

F32 = mybir.dt.float32
AX = mybir.AxisListType
ALU = mybir.AluOpType
ACTF = mybir.ActivationFunctionType

D_IN = 1024
D_OUT = 768
NE = 16
NSEL = 60   # 15 tree nodes * 4 selectors
NLEAF = 64  # 16 experts * 4 selectors
NROWS = NSEL + NLEAF  # 124
NCORES = 8
B_FULL = 8192


def build_nc(b_local=1024, loop_reps=None, num_devices=NCORES):
    nc = bacc.Bacc(
        "TRN2", target_bir_lowering=False, debug=False, num_devices=num_devices
    )
    h_ap = nc.dram_tensor("h", [b_local, D_OUT, NE], F32, kind="ExternalInput").ap()
    x_ap = nc.dram_tensor("x", [b_local, D_IN], F32, kind="ExternalInput").ap()
    selw_ap = nc.dram_tensor("sel_w", [NSEL, D_IN], F32, kind="ExternalInput").ap()
    leafw_ap = nc.dram_tensor("leaf_w", [NLEAF, D_IN], F32, kind="ExternalInput").ap()
    leafb_ap = nc.dram_tensor("leaf_b", [1, NLEAF], F32, kind="ExternalInput").ap()
    y_ap = nc.dram_tensor("y_out", [b_local, D_OUT], F32, kind="ExternalOutput").ap()
    w_ap = nc.dram_tensor("w_out", [b_local, NE], F32, kind="ExternalOutput").ap()
    ent_ap = nc.dram_tensor("ent_out", [b_local, 1], F32, kind="ExternalOutput").ap()

    aps = dict(h=h_ap, x=x_ap, sel_w=selw_ap, leaf_w=leafw_ap, leaf_b=leafb_ap,
               y_out=y_ap, w_out=w_ap, ent_out=ent_ap)

    with tile.TileContext(nc) as tc:
        with ExitStack() as ctx:
            if loop_reps is not None:
                with tc.For_i(0, loop_reps, 1):
                    _body(ctx, tc, nc, aps, b_local)
            else:
                _body(ctx, tc, nc, aps, b_local)

    nc.compile()
    return nc


def _body(ctx, tc, nc, aps, b_local):
    NT = b_local // 128     # 128-sample batch tiles
    NH = b_local // 512     # 512-wide matmul column groups

    const = ctx.enter_context(tc.tile_pool(name="const", bufs=1))
    tpsum = ctx.enter_context(tc.tile_pool(name="tpsum", bufs=4, space="PSUM"))
    rpsum = ctx.enter_context(tc.tile_pool(name="rpsum", bufs=2, space="PSUM"))
    xpool = ctx.enter_context(tc.tile_pool(name="xload", bufs=2))
    hpool = ctx.enter_context(tc.tile_pool(name="hload", bufs=2))
    ypool = ctx.enter_context(tc.tile_pool(name="y", bufs=3))
    small = ctx.enter_context(tc.tile_pool(name="small", bufs=3))
    wpool = ctx.enter_context(tc.tile_pool(name="wnorm", bufs=3))

    # ---- constants -----------------------------------------------------
    identity = const.tile([128, 128], F32)
    masks.make_identity(nc, identity[:])

    ones1 = const.tile([1, 128], F32)
    nc.gpsimd.memset(ones1[:], 1.0)

    zero_b = const.tile([128, 1], F32)
    nc.gpsimd.memset(zero_b[:], 0.0)
    eps_b = const.tile([128, 1], F32)
    nc.gpsimd.memset(eps_b[:], 1e-6)

    leafb_sb = const.tile([1, NLEAF], F32)
    nc.gpsimd.dma_start(leafb_sb[:], aps["leaf_b"][:])
    bp = rpsum.tile([128, NLEAF], F32, tag="bcast", bufs=1)
    nc.tensor.matmul(bp[:], ones1[:], leafb_sb[:], start=True, stop=True)
    Bb = const.tile([128, NLEAF], F32)
    nc.scalar.copy(Bb[:], bp[:])

    # ---- weights: load natural, PE-transpose to (d, row) chunks --------
    Wnat = const.tile([128, D_IN], F32)
    nc.gpsimd.dma_start(Wnat[0:NSEL, :], aps["sel_w"][:])
    nc.gpsimd.dma_start(Wnat[NSEL:NROWS, :], aps["leaf_w"][:])
    WT = const.tile([128, 8, NROWS], F32)
    for c in range(8):
        pt = tpsum.tile([128, NROWS], F32, tag="tp")
        nc.tensor.transpose(
            pt[:], Wnat[0:NROWS, c * 128:(c + 1) * 128], identity[0:NROWS, 0:NROWS]
        )
        nc.scalar.copy(WT[:, c, :], pt[:])

    # ---- x: load natural, PE-transpose to xT[(d_chunk), d_in_chunk, b] -
    xT = const.tile([128, 8, b_local], F32)
    for bt in range(NT):
        xt = xpool.tile([128, D_IN], F32)
        nc.gpsimd.dma_start(xt[:], aps["x"][bt * 128:(bt + 1) * 128, :])
        for c in range(8):
            pt = tpsum.tile([128, 128], F32, tag="tp")
            nc.tensor.transpose(pt[:], xt[:, c * 128:(c + 1) * 128], identity[:])
            if c % 2 == 0:
                nc.scalar.copy(xT[:, c, bt * 128:(bt + 1) * 128], pt[:])
            else:
                nc.vector.tensor_copy(xT[:, c, bt * 128:(bt + 1) * 128], pt[:])

    # ---- routing matmul: R^T (124, b_local) ----------------------------
    r_sb = const.tile([NROWS, b_local], F32)
    for hf in range(NH):
        rp = rpsum.tile([NROWS, 512], F32, tag="rp")
        for c in range(8):
            nc.tensor.matmul(
                rp[:], WT[:, c, :], xT[:, c, hf * 512:(hf + 1) * 512],
                start=(c == 0), stop=(c == 7),
            )
        nc.scalar.copy(r_sb[:, hf * 512:(hf + 1) * 512], rp[:])

    # ---- per-batch-tile routing + aggregation --------------------------
    for bt in range(NT):
        bs = slice(bt * 128, (bt + 1) * 128)

        # h stream (prefetched by the DMA queue, gated only by pool bufs)
        ht = hpool.tile([128, D_OUT, NE], F32)
        nc.sync.dma_start(ht[:], aps["h"][bs])

        # transpose routing result back to (b, slot)
        pt = tpsum.tile([128, NROWS], F32, tag="tp")
        nc.tensor.transpose(pt[:], r_sb[:, bs], identity[0:NROWS, 0:NROWS])
        rt = small.tile([128, NROWS], F32, tag="rt")
        nc.scalar.copy(rt[:], pt[:])

        t_sl = rt[:, 0:NSEL]
        a_sl = rt[:, NSEL:NROWS]

        # smooth-step: g = poly(clamp(t, -.5, .5)), gm = 1 - g
        tcl = small.tile([128, NSEL], F32, tag="tcl")
        nc.vector.tensor_scalar(tcl[:], t_sl, -0.5, 0.5, ALU.max, ALU.min)
        u = small.tile([128, NSEL], F32, tag="u")
        nc.vector.tensor_tensor(u[:], tcl[:], tcl[:], ALU.mult)
        v = small.tile([128, NSEL], F32, tag="v")
        nc.vector.tensor_scalar(v[:], u[:], -2.0, 1.5, ALU.mult, ALU.add)
        g0 = small.tile([128, NSEL], F32, tag="g0")
        nc.vector.tensor_tensor(g0[:], tcl[:], v[:], ALU.mult)
        g = small.tile([128, NSEL], F32, tag="g")
        nc.vector.tensor_scalar(g[:], g0[:], 0.5, None, ALU.add)
        gm = small.tile([128, NSEL], F32, tag="gm")
        nc.vector.tensor_scalar(gm[:], g[:], -1.0, 1.0, ALU.mult, ALU.add)

        # tree expansion (level-order); node j at depth d -> cols 4*(start+j)
        g3 = g[:].rearrange("p (n k) -> p n k", k=4)
        gm3 = gm[:].rearrange("p (n k) -> p n k", k=4)
        lv1 = small.tile([128, 2, 4], F32, tag="lv1")
        nc.vector.tensor_copy(lv1[:, 0, :], g3[:, 0, :])
        nc.vector.tensor_copy(lv1[:, 1, :], gm3[:, 0, :])
        lv2 = small.tile([128, 2, 2, 4], F32, tag="lv2")
        nc.vector.tensor_tensor(lv2[:, :, 0, :], lv1[:, :, :], g3[:, 1:3, :], ALU.mult)
        nc.vector.tensor_tensor(lv2[:, :, 1, :], lv1[:, :, :], gm3[:, 1:3, :], ALU.mult)
        lv3 = small.tile([128, 4, 2, 4], F32, tag="lv3")
        lv2f = lv2[:].rearrange("p a b k -> p (a b) k")
        nc.vector.tensor_tensor(lv3[:, :, 0, :], lv2f, g3[:, 3:7, :], ALU.mult)
        nc.vector.tensor_tensor(lv3[:, :, 1, :], lv2f, gm3[:, 3:7, :], ALU.mult)
        P = small.tile([128, 8, 2, 4], F32, tag="P")
        lv3f = lv3[:].rearrange("p a b k -> p (a b) k")
        nc.vector.tensor_tensor(P[:, :, 0, :], lv3f, g3[:, 7:15, :], ALU.mult)
        nc.vector.tensor_tensor(P[:, :, 1, :], lv3f, gm3[:, 7:15, :], ALU.mult)
        Pf = P[:].rearrange("p a b k -> p (a b k)")   # (128, 64)

        # leaf weights: w_un = (P + 1e-6) * exp(a + leaf_b) * (P >= 1e-5)
        ab = small.tile([128, NLEAF], F32, tag="ab")
        nc.vector.tensor_tensor(ab[:], a_sl, Bb[:], ALU.add)
        sexp = small.tile([128, NLEAF], F32, tag="sexp")
        nc.scalar.activation(sexp[:], ab[:], ACTF.Exp, bias=zero_b[:])
        mask = small.tile([128, NLEAF], F32, tag="mask")
        nc.vector.tensor_scalar(mask[:], Pf, 1e-5, None, ALU.is_ge)
        tmp = small.tile([128, NLEAF], F32, tag="tmp")
        nc.vector.scalar_tensor_tensor(tmp[:], Pf, 1e-6, sexp[:], ALU.add, ALU.mult)
        w_un = small.tile([128, NLEAF], F32, tag="w_un")
        nc.vector.tensor_tensor(w_un[:], tmp[:], mask[:], ALU.mult)

        Z = small.tile([128, 1], F32, tag="Z")
        nc.vector.tensor_reduce(Z[:], w_un[:], axis=AX.X, op=ALU.add)
        rZ = small.tile([128, 1], F32, tag="rZ")
        nc.vector.reciprocal(rZ[:], Z[:])
        w_eun = small.tile([128, NE], F32, tag="w_eun")
        w_un3 = w_un[:].rearrange("p (e k) -> p e k", k=4)
        nc.vector.tensor_reduce(w_eun[:], w_un3, axis=AX.X, op=ALU.add)
        w_norm = wpool.tile([128, NE], F32)
        nc.vector.tensor_scalar(w_norm[:], w_eun[:], rZ[:], None, ALU.mult)
        nc.gpsimd.dma_start(aps["w_out"][bs], w_norm[:])

        # entropy: ent = -sum((pc + 1e-6) * ln(pc + 1e-6)), pc = max(P, 1e-6)
        pc = small.tile([128, NLEAF], F32, tag="pc")
        nc.vector.tensor_scalar(pc[:], Pf, 1e-6, None, ALU.max)
        lg = small.tile([128, NLEAF], F32, tag="lg")
        nc.scalar.activation(lg[:], pc[:], ACTF.Ln, bias=eps_b[:])
        prod = small.tile([128, NLEAF], F32, tag="prod")
        nc.vector.scalar_tensor_tensor(prod[:], pc[:], 1e-6, lg[:], ALU.add, ALU.mult)
        ent = small.tile([128, 1], F32, tag="ent")
        nc.vector.tensor_reduce(ent[:], prod[:], axis=AX.X, op=ALU.add, negate=True)
        nc.gpsimd.dma_start(aps["ent_out"][bs], ent[:])

        # y = sum_e h[:, :, e] * w_norm[:, e]  (fused mult-add chain)
        yk = ypool.tile([128, D_OUT], F32, tag="yk")
        nc.vector.tensor_scalar(yk[:], ht[:, :, 0], w_norm[:, 0:1], None, ALU.mult)
        for e in range(1, NE):
            yn = ypool.tile([128, D_OUT], F32, tag="yk")
            nc.vector.scalar_tensor_tensor(
                yn[:], ht[:, :, e], w_norm[:, e:e + 1], yk[:], ALU.mult, ALU.add
            )
            yk = yn
        nc.scalar.dma_start(aps["y_out"][bs], yk[:])


_NC_CACHE = {}


def _get_nc():
    if "nc" not in _NC_CACHE:
        _NC_CACHE["nc"] = build_nc(B_FULL // NCORES, None, NCORES)
    return _NC_CACHE["nc"]


def kernel(h, x, sel_w, leaf_w, leaf_b):
    from concourse.bass_utils import run_bass_kernel_spmd

    nc = _get_nc()
    BL = B_FULL // NCORES
    selw2 = np.ascontiguousarray(np.asarray(sel_w, np.float32).reshape(NSEL, D_IN))
    leafw2 = np.ascontiguousarray(np.asarray(leaf_w, np.float32).reshape(NLEAF, D_IN))
    leafb2 = np.ascontiguousarray(np.asarray(leaf_b, np.float32).reshape(1, NLEAF))
    h = np.asarray(h, np.float32)
    x = np.asarray(x, np.float32)
    in_maps = []
    for i in range(NCORES):
        sl = slice(i * BL, (i + 1) * BL)
        in_maps.append({
            "h": np.ascontiguousarray(h[sl]),
            "x": np.ascontiguousarray(x[sl]),
            "sel_w": selw2,
            "leaf_w": leafw2,
            "leaf_b": leafb2,
        })
    res = run_bass_kernel_spmd(nc, in_maps, core_ids=list(range(NCORES)))
    y = np.concatenate([res.results[i]["y_out"] for i in range(NCORES)], 0)
    w = np.concatenate([res.results[i]["w_out"] for i in range(NCORES)], 0)
    ent = np.concatenate([res.results[i]["ent_out"] for i in range(NCORES)], 0)[:, 0]

    s = w < 1e-5
    soft = w.mean(0, dtype=np.float64).astype(np.float32)
    hard = (1.0 - s).mean(0, dtype=np.float64).astype(np.float32)
    s_concat = s.astype(np.float32)[:, None, None, :]
    reg = np.float32(0.01 * ent.mean(dtype=np.float64))
    return (y, soft, hard, s_concat, reg)


# revision 9
# speedup vs baseline: 1.5057x; 1.5057x over previous
"""COMET MoE-routing kernel for one TRN2 chip (8 NeuronCores).

Strategy: pure data parallelism over the batch (8192 -> 8 x 1024). Per core:
  - routing matmul  R^T = [sel_w; leaf_w] @ x^T  on the TensorEngine (fp32,
    contraction over d_in in 8 chunks of 128, inputs PE-transposed on chip)
  - smooth-step gates, binary-tree path products, masked softmax-weight
    computation on VectorE/ScalarE in (batch-partition, slot-free) layout
  - h is streamed in 16 chunks (128 samples x 384 d_out x 16 experts),
    cast f32->bf16 on GpSimd, and reduced over experts on the TensorEngine:
    y_psum += diag(w_norm[:, e]) @ h_bf16[:, :, e], accumulating all 16
    experts into PSUM (diagonal matmuls scale per-sample, PSUM sums experts)
  - per-sample outputs y (B,768), w_norm (B,16), entropy (B,1); the tiny
    cross-batch means / thresholds / reg scalar are finished on the host.
No collectives are needed: every cross-batch reduction is 16 floats per core.
"""
from contextlib import ExitStack

import numpy as np

import concourse.bass as bass
import concourse.tile as tile
from concourse import bacc, masks, mybir

F32 = mybir.dt.float32
BF16 = mybir.dt.bfloat16
AX = mybir.AxisListType
ALU = mybir.AluOpType
ACTF = mybir.ActivationFunctionType

D_IN = 1024
D_OUT = 768
DH = D_OUT // 2          # 384: d_out half processed per h chunk
NE = 16
NSEL = 60   # 15 tree nodes * 4 selectors
NLEAF = 64  # 16 experts * 4 selectors
NROWS = NSEL + NLEAF  # 124
NCORES = 8
B_FULL = 8192


def build_nc(b_local=1024, loop_reps=None, num_devices=NCORES):
    nc = bacc.Bacc(
        "TRN2", target_bir_lowering=False, debug=False, num_devices=num_devices
    )
    h_ap = nc.dram_tensor("h", [b_local, D_OUT, NE], F32, kind="ExternalInput").ap()
    x_ap = nc.dram_tensor("x", [b_local, D_IN], F32, kind="ExternalInput").ap()
    selw_ap = nc.dram_tensor("sel_w", [NSEL, D_IN], F32, kind="ExternalInput").ap()
    leafw_ap = nc.dram_tensor("leaf_w", [NLEAF, D_IN], F32, kind="ExternalInput").ap()
    leafb_ap = nc.dram_tensor("leaf_b", [1, NLEAF], F32, kind="ExternalInput").ap()
    y_ap = nc.dram_tensor("y_out", [b_local, D_OUT], F32, kind="ExternalOutput").ap()
    w_ap = nc.dram_tensor("w_out", [b_local, NE], F32, kind="ExternalOutput").ap()
    ent_ap = nc.dram_tensor("ent_out", [b_local, 1], F32, kind="ExternalOutput").ap()

    aps = dict(h=h_ap, x=x_ap, sel_w=selw_ap, leaf_w=leafw_ap, leaf_b=leafb_ap,
               y_out=y_ap, w_out=w_ap, ent_out=ent_ap)

    with tile.TileContext(nc) as tc:
        with ExitStack() as ctx:
            if loop_reps is not None:
                with tc.For_i(0, loop_reps, 1):
                    _body(ctx, tc, nc, aps, b_local)
            else:
                _body(ctx, tc, nc, aps, b_local)

    nc.compile()
    return nc


def _body(ctx, tc, nc, aps, b_local):
    NT = b_local // 128     # 128-sample batch tiles
    NH = b_local // 512     # 512-wide matmul column groups

    const = ctx.enter_context(tc.tile_pool(name="const", bufs=1))
    tpsum = ctx.enter_context(tc.tile_pool(name="tpsum", bufs=2, space="PSUM"))
    rpsum = ctx.enter_context(tc.tile_pool(name="rpsum", bufs=1, space="PSUM"))
    ypsum = ctx.enter_context(tc.tile_pool(name="ypsum", bufs=4, space="PSUM"))
    xpool = ctx.enter_context(tc.tile_pool(name="xload", bufs=4))
    hpool = ctx.enter_context(tc.tile_pool(name="hload", bufs=2))
    hbpool = ctx.enter_context(tc.tile_pool(name="hbf", bufs=3))
    dpool = ctx.enter_context(tc.tile_pool(name="diag", bufs=2))
    ypool = ctx.enter_context(tc.tile_pool(name="y", bufs=3))
    small = ctx.enter_context(tc.tile_pool(name="small", bufs=3))
    wpool = ctx.enter_context(tc.tile_pool(name="wnorm", bufs=3))

    # ---- x first on the sync DMA queue (it gates all routing) ----------
    xnat = []
    for bt in range(NT):
        xt = xpool.tile([128, D_IN], F32, tag="xt")
        nc.sync.dma_start(xt[:], aps["x"][bt * 128:(bt + 1) * 128, :])
        xnat.append(xt)

    # ---- constants -----------------------------------------------------
    identity = const.tile([128, 128], F32)
    masks.make_identity(nc, identity[:])
    identity_bf = const.tile([128, 128], BF16)
    masks.make_identity(nc, identity_bf[:])

    ones1 = const.tile([1, 128], F32)
    nc.gpsimd.memset(ones1[:], 1.0)

    zero_b = const.tile([128, 1], F32)
    nc.gpsimd.memset(zero_b[:], 0.0)
    eps_b = const.tile([128, 1], F32)
    nc.gpsimd.memset(eps_b[:], 1e-6)

    leafb_sb = const.tile([1, NLEAF], F32)
    nc.sync.dma_start(leafb_sb[:], aps["leaf_b"][:])
    bp = rpsum.tile([128, NLEAF], F32, tag="rp")
    nc.tensor.matmul(bp[:], ones1[:], leafb_sb[:], start=True, stop=True)
    Bb = const.tile([128, NLEAF], F32)
    nc.scalar.copy(Bb[:], bp[:])

    # ---- weights: load natural, PE-transpose to (d, row) chunks --------
    Wnat = const.tile([128, D_IN], F32)
    nc.sync.dma_start(Wnat[0:NSEL, :], aps["sel_w"][:])
    nc.sync.dma_start(Wnat[NSEL:NROWS, :], aps["leaf_w"][:])
    WT = const.tile([128, 8, NROWS], F32)
    for c in range(8):
        pt = tpsum.tile([128, NROWS], F32, tag="tp")
        nc.tensor.transpose(
            pt[:], Wnat[0:NROWS, c * 128:(c + 1) * 128], identity[0:NROWS, 0:NROWS]
        )
        nc.scalar.copy(WT[:, c, :], pt[:])

    # ---- x: PE-transpose to xT[d_chunk][d_in_chunk, b] -----------------
    xT = const.tile([128, 8, b_local], F32)
    for bt in range(NT):
        for c in range(8):
            pt = tpsum.tile([128, 128], F32, tag="tp")
            nc.tensor.transpose(pt[:], xnat[bt][:, c * 128:(c + 1) * 128], identity[:])
            if c % 2 == 0:
                nc.scalar.copy(xT[:, c, bt * 128:(bt + 1) * 128], pt[:])
            else:
                nc.vector.tensor_copy(xT[:, c, bt * 128:(bt + 1) * 128], pt[:])

    # ---- h stream on the sync queue (after all x) ----------------------
    hts = {}
    for bt in range(NT):
        for dh in range(2):
            ht = hpool.tile([128, DH, NE], F32, tag="ht")
            nc.sync.dma_start(
                ht[:], aps["h"][bt * 128:(bt + 1) * 128, dh * DH:(dh + 1) * DH, :]
            )
            hts[(bt, dh)] = ht

    # ---- routing matmul: R^T (124, b_local) ----------------------------
    r_sb = const.tile([NROWS, b_local], F32)
    for hf in range(NH):
        rp = rpsum.tile([NROWS, 512], F32, tag="rp")
        for c in range(8):
            nc.tensor.matmul(
                rp[:], WT[:, c, :], xT[:, c, hf * 512:(hf + 1) * 512],
                start=(c == 0), stop=(c == 7),
            )
        nc.scalar.copy(r_sb[:, hf * 512:(hf + 1) * 512], rp[:])

    # ---- per-batch-tile routing + aggregation --------------------------
    for bt in range(NT):
        bs = slice(bt * 128, (bt + 1) * 128)

        # transpose routing result back to (b, slot)
        pt = tpsum.tile([128, NROWS], F32, tag="tp")
        nc.tensor.transpose(pt[:], r_sb[:, bs], identity[0:NROWS, 0:NROWS])
        rt = small.tile([128, NROWS], F32, tag="rt")
        nc.scalar.copy(rt[:], pt[:])

        t_sl = rt[:, 0:NSEL]
        a_sl = rt[:, NSEL:NROWS]

        # smooth-step: g = poly(clamp(t, -.5, .5)), gm = 1 - g
        tcl = small.tile([128, NSEL], F32, tag="tcl")
        nc.vector.tensor_scalar(tcl[:], t_sl, -0.5, 0.5, ALU.max, ALU.min)
        u = small.tile([128, NSEL], F32, tag="u")
        nc.vector.tensor_tensor(u[:], tcl[:], tcl[:], ALU.mult)
        v = small.tile([128, NSEL], F32, tag="v")
        nc.vector.tensor_scalar(v[:], u[:], -2.0, 1.5, ALU.mult, ALU.add)
        g0 = small.tile([128, NSEL], F32, tag="g0")
        nc.vector.tensor_tensor(g0[:], tcl[:], v[:], ALU.mult)
        g = small.tile([128, NSEL], F32, tag="g")
        nc.vector.tensor_scalar(g[:], g0[:], 0.5, None, ALU.add)
        gm = small.tile([128, NSEL], F32, tag="gm")
        nc.vector.tensor_scalar(gm[:], g[:], -1.0, 1.0, ALU.mult, ALU.add)

        # tree expansion (level-order); node j at depth d -> cols 4*(start+j)
        g3 = g[:].rearrange("p (n k) -> p n k", k=4)
        gm3 = gm[:].rearrange("p (n k) -> p n k", k=4)
        lv1 = small.tile([128, 2, 4], F32, tag="lv1")
        nc.vector.tensor_copy(lv1[:, 0, :], g3[:, 0, :])
        nc.vector.tensor_copy(lv1[:, 1, :], gm3[:, 0, :])
        lv2 = small.tile([128, 2, 2, 4], F32, tag="lv2")
        nc.vector.tensor_tensor(lv2[:, :, 0, :], lv1[:, :, :], g3[:, 1:3, :], ALU.mult)
        nc.vector.tensor_tensor(lv2[:, :, 1, :], lv1[:, :, :], gm3[:, 1:3, :], ALU.mult)
        lv3 = small.tile([128, 4, 2, 4], F32, tag="lv3")
        lv2f = lv2[:].rearrange("p a b k -> p (a b) k")
        nc.vector.tensor_tensor(lv3[:, :, 0, :], lv2f, g3[:, 3:7, :], ALU.mult)
        nc.vector.tensor_tensor(lv3[:, :, 1, :], lv2f, gm3[:, 3:7, :], ALU.mult)
        P = small.tile([128, 8, 2, 4], F32, tag="P")
        lv3f = lv3[:].rearrange("p a b k -> p (a b) k")
        nc.vector.tensor_tensor(P[:, :, 0, :], lv3f, g3[:, 7:15, :], ALU.mult)
        nc.vector.tensor_tensor(P[:, :, 1, :], lv3f, gm3[:, 7:15, :], ALU.mult)
        Pf = P[:].rearrange("p a b k -> p (a b k)")   # (128, 64)

        # leaf weights: w_un = (P + 1e-6) * exp(a + leaf_b) * (P >= 1e-5)
        ab = small.tile([128, NLEAF], F32, tag="ab")
        nc.vector.tensor_tensor(ab[:], a_sl, Bb[:], ALU.add)
        sexp = small.tile([128, NLEAF], F32, tag="sexp")
        nc.scalar.activation(sexp[:], ab[:], ACTF.Exp, bias=zero_b[:])
        mask = small.tile([128, NLEAF], F32, tag="mask")
        nc.vector.tensor_scalar(mask[:], Pf, 1e-5, None, ALU.is_ge)
        tmp = small.tile([128, NLEAF], F32, tag="tmp")
        nc.vector.scalar_tensor_tensor(tmp[:], Pf, 1e-6, sexp[:], ALU.add, ALU.mult)
        w_un = small.tile([128, NLEAF], F32, tag="w_un")
        nc.vector.tensor_tensor(w_un[:], tmp[:], mask[:], ALU.mult)

        Z = small.tile([128, 1], F32, tag="Z")
        nc.vector.tensor_reduce(Z[:], w_un[:], axis=AX.X, op=ALU.add)
        rZ = small.tile([128, 1], F32, tag="rZ")
        nc.vector.reciprocal(rZ[:], Z[:])
        w_eun = small.tile([128, NE], F32, tag="w_eun")
        w_un3 = w_un[:].rearrange("p (e k) -> p e k", k=4)
        nc.vector.tensor_reduce(w_eun[:], w_un3, axis=AX.X, op=ALU.add)
        w_norm = wpool.tile([128, NE], F32)
        nc.vector.tensor_scalar(w_norm[:], w_eun[:], rZ[:], None, ALU.mult)
        nc.gpsimd.dma_start(aps["w_out"][bs], w_norm[:])

        # entropy: ent = -sum((pc + 1e-6) * ln(pc + 1e-6)), pc = max(P, 1e-6)
        pc = small.tile([128, NLEAF], F32, tag="pc")
        nc.vector.tensor_scalar(pc[:], Pf, 1e-6, None, ALU.max)
        lg = small.tile([128, NLEAF], F32, tag="lg")
        nc.scalar.activation(lg[:], pc[:], ACTF.Ln, bias=eps_b[:])
        prod = small.tile([128, NLEAF], F32, tag="prod")
        nc.vector.scalar_tensor_tensor(prod[:], pc[:], 1e-6, lg[:], ALU.add, ALU.mult)
        ent = small.tile([128, 1], F32, tag="ent")
        nc.vector.tensor_reduce(ent[:], prod[:], axis=AX.X, op=ALU.add, negate=True)
        nc.gpsimd.dma_start(aps["ent_out"][bs], ent[:])

        # per-sample expert weights as bf16 diagonals for the PE reduction
        diag = dpool.tile([128, NE, 128], BF16)
        for e in range(NE):
            nc.vector.tensor_scalar(
                diag[:, e, :], identity_bf[:], w_norm[:, e:e + 1], None, ALU.mult
            )

        # y[:, dh] = sum_e diag(w[:, e]) @ h_bf16[:, :, e]  (PSUM accumulate)
        for dh in range(2):
            hb = hbpool.tile([128, DH, NE], BF16, tag="hb")
            nc.vector.tensor_copy(hb[:], hts[(bt, dh)][:])
            yp = ypsum.tile([128, DH], F32, tag="yp")
            for e in range(NE):
                nc.tensor.matmul(
                    yp[:], diag[:, e, :], hb[:, :, e],
                    start=(e == 0), stop=(e == NE - 1),
                )
            ysb = ypool.tile([128, DH], F32, tag="ysb")
            nc.scalar.copy(ysb[:], yp[:])
            nc.gpsimd.dma_start(aps["y_out"][bs, dh * DH:(dh + 1) * DH], ysb[:])


_NC_CACHE = {}


def _get_nc():
    if "nc" not in _NC_CACHE:
        _NC_CACHE["nc"] = build_nc(B_FULL // NCORES, None, NCORES)
    return _NC_CACHE["nc"]


def kernel(h, x, sel_w, leaf_w, leaf_b):
    from concourse.bass_utils import run_bass_kernel_spmd

    nc = _get_nc()
    BL = B_FULL // NCORES
    selw2 = np.ascontiguousarray(np.asarray(sel_w, np.float32).reshape(NSEL, D_IN))
    leafw2 = np.ascontiguousarray(np.asarray(leaf_w, np.float32).reshape(NLEAF, D_IN))
    leafb2 = np.ascontiguousarray(np.asarray(leaf_b, np.float32).reshape(1, NLEAF))
    h = np.asarray(h, np.float32)
    x = np.asarray(x, np.float32)
    in_maps = []
    for i in range(NCORES):
        sl = slice(i * BL, (i + 1) * BL)
        in_maps.append({
            "h": np.ascontiguousarray(h[sl]),
            "x": np.ascontiguousarray(x[sl]),
            "sel_w": selw2,
            "leaf_w": leafw2,
            "leaf_b": leafb2,
        })
    res = run_bass_kernel_spmd(nc, in_maps, core_ids=list(range(NCORES)))
    y = np.concatenate([res.results[i]["y_out"] for i in range(NCORES)], 0)
    w = np.concatenate([res.results[i]["w_out"] for i in range(NCORES)], 0)
    ent = np.concatenate([res.results[i]["ent_out"] for i in range(NCORES)], 0)[:, 0]

    s = w < 1e-5
    soft = w.mean(0, dtype=np.float64).astype(np.float32)
    hard = (1.0 - s).mean(0, dtype=np.float64).astype(np.float32)
    s_concat = s.astype(np.float32)[:, None, None, :]
    reg = np.float32(0.01 * ent.mean(dtype=np.float64))
    return (y, soft, hard, s_concat, reg)


# revision 10
# speedup vs baseline: 1.9535x; 1.2973x over previous
"""COMET MoE-routing kernel for one TRN2 chip (8 NeuronCores).

Strategy: pure data parallelism over the batch (8192 -> 8 x 1024). Per core:
  - routing matmul  R^T = [sel_w; leaf_w] @ x^T  on the TensorEngine (fp32,
    contraction over d_in in 8 chunks of 128, inputs PE-transposed on chip)
  - smooth-step gates, binary-tree path products, masked softmax-weight
    computation on VectorE/ScalarE in (batch-partition, slot-free) layout
  - h is streamed in 16 chunks (128 samples x 384 d_out x 16 experts),
    cast f32->bf16 on GpSimd, and reduced over experts on the TensorEngine:
    y_psum += diag(w_norm[:, e]) @ h_bf16[:, :, e], accumulating all 16
    experts into PSUM (diagonal matmuls scale per-sample, PSUM sums experts)
  - per-sample outputs y (B,768), w_norm (B,16), entropy (B,1); the tiny
    cross-batch means / thresholds / reg scalar are finished on the host.
No collectives are needed: every cross-batch reduction is 16 floats per core.
"""
from contextlib import ExitStack

import numpy as np

import concourse.bass as bass
import concourse.tile as tile
from concourse import bacc, masks, mybir

F32 = mybir.dt.float32
BF16 = mybir.dt.bfloat16
AX = mybir.AxisListType
ALU = mybir.AluOpType
ACTF = mybir.ActivationFunctionType

D_IN = 1024
D_OUT = 768
DH = D_OUT // 2          # 384: d_out half processed per h chunk
NE = 16
NSEL = 60   # 15 tree nodes * 4 selectors
NLEAF = 64  # 16 experts * 4 selectors
NROWS = NSEL + NLEAF  # 124
NCORES = 8
B_FULL = 8192


def build_nc(b_local=1024, loop_reps=None, num_devices=NCORES, stages="full"):
    nc = bacc.Bacc(
        "TRN2", target_bir_lowering=False, debug=False, num_devices=num_devices
    )
    h_ap = nc.dram_tensor("h", [b_local, D_OUT, NE], F32, kind="ExternalInput").ap()
    x_ap = nc.dram_tensor("x", [b_local, D_IN], F32, kind="ExternalInput").ap()
    selw_ap = nc.dram_tensor("sel_w", [NSEL, D_IN], F32, kind="ExternalInput").ap()
    leafw_ap = nc.dram_tensor("leaf_w", [NLEAF, D_IN], F32, kind="ExternalInput").ap()
    leafb_ap = nc.dram_tensor("leaf_b", [1, NLEAF], F32, kind="ExternalInput").ap()
    y_ap = nc.dram_tensor("y_out", [b_local, D_OUT], F32, kind="ExternalOutput").ap()
    w_ap = nc.dram_tensor("w_out", [b_local, NE], F32, kind="ExternalOutput").ap()
    ent_ap = nc.dram_tensor("ent_out", [b_local, 1], F32, kind="ExternalOutput").ap()

    aps = dict(h=h_ap, x=x_ap, sel_w=selw_ap, leaf_w=leafw_ap, leaf_b=leafb_ap,
               y_out=y_ap, w_out=w_ap, ent_out=ent_ap)

    with tile.TileContext(nc) as tc:
        with ExitStack() as ctx:
            if loop_reps is not None:
                with tc.For_i(0, loop_reps, 1):
                    _body(ctx, tc, nc, aps, b_local, stages)
            else:
                _body(ctx, tc, nc, aps, b_local, stages)

    nc.compile()
    return nc


def _body(ctx, tc, nc, aps, b_local, stages="full"):
    NT = b_local // 128     # 128-sample batch tiles
    NH = b_local // 512     # 512-wide matmul column groups

    const = ctx.enter_context(tc.tile_pool(name="const", bufs=1))
    tpsum = ctx.enter_context(tc.tile_pool(name="tpsum", bufs=2, space="PSUM"))
    rpsum = ctx.enter_context(tc.tile_pool(name="rpsum", bufs=1, space="PSUM"))
    ypsum = ctx.enter_context(tc.tile_pool(name="ypsum", bufs=4, space="PSUM"))
    xpool = ctx.enter_context(tc.tile_pool(name="xload", bufs=4))
    hpool = ctx.enter_context(tc.tile_pool(name="hload", bufs=2))
    hbpool = ctx.enter_context(tc.tile_pool(name="hbf", bufs=3))
    dpool = ctx.enter_context(tc.tile_pool(name="diag", bufs=2))
    ypool = ctx.enter_context(tc.tile_pool(name="y", bufs=3))
    small = ctx.enter_context(tc.tile_pool(name="small", bufs=3))
    wpool = ctx.enter_context(tc.tile_pool(name="wnorm", bufs=3))

    # ---- x first on the sync DMA queue (it gates all routing) ----------
    xnat = []
    for bt in range(NT):
        xt = xpool.tile([128, D_IN], F32, tag="xt")
        nc.sync.dma_start(xt[:], aps["x"][bt * 128:(bt + 1) * 128, :])
        xnat.append(xt)

    # ---- constants -----------------------------------------------------
    identity = const.tile([128, 128], F32)
    masks.make_identity(nc, identity[:])
    identity_bf = const.tile([128, 128], BF16)
    masks.make_identity(nc, identity_bf[:])

    ones1 = const.tile([1, 128], F32)
    nc.gpsimd.memset(ones1[:], 1.0)

    zero_b = const.tile([128, 1], F32)
    nc.gpsimd.memset(zero_b[:], 0.0)
    eps_b = const.tile([128, 1], F32)
    nc.gpsimd.memset(eps_b[:], 1e-6)

    leafb_sb = const.tile([1, NLEAF], F32)
    nc.sync.dma_start(leafb_sb[:], aps["leaf_b"][:])
    bp = rpsum.tile([128, NLEAF], F32, tag="rp")
    nc.tensor.matmul(bp[:], ones1[:], leafb_sb[:], start=True, stop=True)
    Bb = const.tile([128, NLEAF], F32)
    nc.scalar.copy(Bb[:], bp[:])

    # ---- weights: load natural, PE-transpose to (d, row) chunks --------
    Wnat = const.tile([128, D_IN], F32)
    nc.sync.dma_start(Wnat[0:NSEL, :], aps["sel_w"][:])
    nc.sync.dma_start(Wnat[NSEL:NROWS, :], aps["leaf_w"][:])
    WT = const.tile([128, 8, NROWS], F32)
    for c in range(8):
        pt = tpsum.tile([128, NROWS], F32, tag="tp")
        nc.tensor.transpose(
            pt[:], Wnat[0:NROWS, c * 128:(c + 1) * 128], identity[0:NROWS, 0:NROWS]
        )
        nc.scalar.copy(WT[:, c, :], pt[:])

    # ---- x: PE-transpose to xT[d_chunk][d_in_chunk, b] -----------------
    xT = const.tile([128, 8, b_local], F32)
    for bt in range(NT):
        for c in range(8):
            pt = tpsum.tile([128, 128], F32, tag="tp")
            nc.tensor.transpose(pt[:], xnat[bt][:, c * 128:(c + 1) * 128], identity[:])
            if c % 2 == 0:
                nc.scalar.copy(xT[:, c, bt * 128:(bt + 1) * 128], pt[:])
            else:
                nc.vector.tensor_copy(xT[:, c, bt * 128:(bt + 1) * 128], pt[:])

    # ---- h stream on the sync queue (after all x) ----------------------
    hts = {}
    for bt in range(NT):
        for dh in range(2):
            ht = hpool.tile([128, DH, NE], F32, tag="ht")
            nc.sync.dma_start(
                ht[:], aps["h"][bt * 128:(bt + 1) * 128, dh * DH:(dh + 1) * DH, :]
            )
            hts[(bt, dh)] = ht

    if stages == "dma":
        fake_y = const.tile([128, DH], F32)
        nc.gpsimd.memset(fake_y[:], 0.5)
        fake_w = const.tile([128, NE], F32)
        nc.gpsimd.memset(fake_w[:], 0.25)
        fake_e = const.tile([128, 1], F32)
        nc.gpsimd.memset(fake_e[:], 0.125)
        for bt in range(NT):
            bs = slice(bt * 128, (bt + 1) * 128)
            nc.gpsimd.dma_start(aps["w_out"][bs], fake_w[:])
            nc.gpsimd.dma_start(aps["ent_out"][bs], fake_e[:])
            for dh in range(2):
                nc.gpsimd.dma_start(aps["y_out"][bs, dh * DH:(dh + 1) * DH], fake_y[:])
        return
    if stages == "ymm":
        # h DMA -> DVE cast -> PE matmuls with constant identity diagonals
        fake_w = const.tile([128, NE], F32)
        nc.gpsimd.memset(fake_w[:], 0.25)
        fake_e = const.tile([128, 1], F32)
        nc.gpsimd.memset(fake_e[:], 0.125)
        for bt in range(NT):
            bs = slice(bt * 128, (bt + 1) * 128)
            nc.gpsimd.dma_start(aps["w_out"][bs], fake_w[:])
            nc.gpsimd.dma_start(aps["ent_out"][bs], fake_e[:])
            for dh in range(2):
                hb = hbpool.tile([128, DH, NE], BF16, tag="hb")
                nc.vector.tensor_copy(hb[:], hts[(bt, dh)][:])
                yp = ypsum.tile([128, DH], F32, tag="yp")
                for e in range(NE):
                    nc.tensor.matmul(
                        yp[:], identity_bf[:], hb[:, :, e],
                        start=(e == 0), stop=(e == NE - 1),
                    )
                ysb = ypool.tile([128, DH], F32, tag="ysb")
                nc.scalar.copy(ysb[:], yp[:])
                nc.gpsimd.dma_start(aps["y_out"][bs, dh * DH:(dh + 1) * DH], ysb[:])
        return

    # ---- routing matmul: R^T (124, b_local) ----------------------------
    r_sb = const.tile([NROWS, b_local], F32)
    for hf in range(NH):
        rp = rpsum.tile([NROWS, 512], F32, tag="rp")
        for c in range(8):
            nc.tensor.matmul(
                rp[:], WT[:, c, :], xT[:, c, hf * 512:(hf + 1) * 512],
                start=(c == 0), stop=(c == 7),
            )
        nc.scalar.copy(r_sb[:, hf * 512:(hf + 1) * 512], rp[:])

    # ---- per-batch-tile routing + aggregation --------------------------
    for bt in range(NT):
        bs = slice(bt * 128, (bt + 1) * 128)

        # transpose routing result back to (b, slot)
        pt = tpsum.tile([128, NROWS], F32, tag="tp")
        nc.tensor.transpose(pt[:], r_sb[:, bs], identity[0:NROWS, 0:NROWS])
        rt = small.tile([128, NROWS], F32, tag="rt")
        nc.scalar.copy(rt[:], pt[:])

        t_sl = rt[:, 0:NSEL]
        a_sl = rt[:, NSEL:NROWS]

        # smooth-step: g = poly(clamp(t, -.5, .5)), gm = 1 - g
        tcl = small.tile([128, NSEL], F32, tag="tcl")
        nc.vector.tensor_scalar(tcl[:], t_sl, -0.5, 0.5, ALU.max, ALU.min)
        u = small.tile([128, NSEL], F32, tag="u")
        nc.vector.tensor_tensor(u[:], tcl[:], tcl[:], ALU.mult)
        v = small.tile([128, NSEL], F32, tag="v")
        nc.vector.tensor_scalar(v[:], u[:], -2.0, 1.5, ALU.mult, ALU.add)
        g0 = small.tile([128, NSEL], F32, tag="g0")
        nc.vector.tensor_tensor(g0[:], tcl[:], v[:], ALU.mult)
        g = small.tile([128, NSEL], F32, tag="g")
        nc.vector.tensor_scalar(g[:], g0[:], 0.5, None, ALU.add)
        gm = small.tile([128, NSEL], F32, tag="gm")
        nc.vector.tensor_scalar(gm[:], g[:], -1.0, 1.0, ALU.mult, ALU.add)

        # tree expansion (level-order); node j at depth d -> cols 4*(start+j)
        g3 = g[:].rearrange("p (n k) -> p n k", k=4)
        gm3 = gm[:].rearrange("p (n k) -> p n k", k=4)
        lv1 = small.tile([128, 2, 4], F32, tag="lv1")
        nc.vector.tensor_copy(lv1[:, 0, :], g3[:, 0, :])
        nc.vector.tensor_copy(lv1[:, 1, :], gm3[:, 0, :])
        lv2 = small.tile([128, 2, 2, 4], F32, tag="lv2")
        nc.vector.tensor_tensor(lv2[:, :, 0, :], lv1[:, :, :], g3[:, 1:3, :], ALU.mult)
        nc.vector.tensor_tensor(lv2[:, :, 1, :], lv1[:, :, :], gm3[:, 1:3, :], ALU.mult)
        lv3 = small.tile([128, 4, 2, 4], F32, tag="lv3")
        lv2f = lv2[:].rearrange("p a b k -> p (a b) k")
        nc.vector.tensor_tensor(lv3[:, :, 0, :], lv2f, g3[:, 3:7, :], ALU.mult)
        nc.vector.tensor_tensor(lv3[:, :, 1, :], lv2f, gm3[:, 3:7, :], ALU.mult)
        P = small.tile([128, 8, 2, 4], F32, tag="P")
        lv3f = lv3[:].rearrange("p a b k -> p (a b) k")
        nc.vector.tensor_tensor(P[:, :, 0, :], lv3f, g3[:, 7:15, :], ALU.mult)
        nc.vector.tensor_tensor(P[:, :, 1, :], lv3f, gm3[:, 7:15, :], ALU.mult)
        Pf = P[:].rearrange("p a b k -> p (a b k)")   # (128, 64)

        # leaf weights: w_un = (P + 1e-6) * exp(a + leaf_b) * (P >= 1e-5)
        ab = small.tile([128, NLEAF], F32, tag="ab")
        nc.vector.tensor_tensor(ab[:], a_sl, Bb[:], ALU.add)
        sexp = small.tile([128, NLEAF], F32, tag="sexp")
        nc.scalar.activation(sexp[:], ab[:], ACTF.Exp, bias=zero_b[:])
        mask = small.tile([128, NLEAF], F32, tag="mask")
        nc.vector.tensor_scalar(mask[:], Pf, 1e-5, None, ALU.is_ge)
        tmp = small.tile([128, NLEAF], F32, tag="tmp")
        nc.vector.scalar_tensor_tensor(tmp[:], Pf, 1e-6, sexp[:], ALU.add, ALU.mult)
        w_un = small.tile([128, NLEAF], F32, tag="w_un")
        nc.vector.tensor_tensor(w_un[:], tmp[:], mask[:], ALU.mult)

        Z = small.tile([128, 1], F32, tag="Z")
        nc.vector.tensor_reduce(Z[:], w_un[:], axis=AX.X, op=ALU.add)
        rZ = small.tile([128, 1], F32, tag="rZ")
        nc.vector.reciprocal(rZ[:], Z[:])
        w_eun = small.tile([128, NE], F32, tag="w_eun")
        w_un3 = w_un[:].rearrange("p (e k) -> p e k", k=4)
        nc.vector.tensor_reduce(w_eun[:], w_un3, axis=AX.X, op=ALU.add)
        w_norm = wpool.tile([128, NE], F32)
        nc.vector.tensor_scalar(w_norm[:], w_eun[:], rZ[:], None, ALU.mult)
        nc.gpsimd.dma_start(aps["w_out"][bs], w_norm[:])

        # entropy: ent = -sum((pc + 1e-6) * ln(pc + 1e-6)), pc = max(P, 1e-6)
        pc = small.tile([128, NLEAF], F32, tag="pc")
        nc.vector.tensor_scalar(pc[:], Pf, 1e-6, None, ALU.max)
        lg = small.tile([128, NLEAF], F32, tag="lg")
        nc.scalar.activation(lg[:], pc[:], ACTF.Ln, bias=eps_b[:])
        prod = small.tile([128, NLEAF], F32, tag="prod")
        nc.vector.scalar_tensor_tensor(prod[:], pc[:], 1e-6, lg[:], ALU.add, ALU.mult)
        ent = small.tile([128, 1], F32, tag="ent")
        nc.vector.tensor_reduce(ent[:], prod[:], axis=AX.X, op=ALU.add, negate=True)
        nc.gpsimd.dma_start(aps["ent_out"][bs], ent[:])

        # per-sample expert weights as bf16 diagonals for the PE reduction
        diag = dpool.tile([128, NE, 128], BF16)
        for e in range(NE):
            nc.vector.tensor_scalar(
                diag[:, e, :], identity_bf[:], w_norm[:, e:e + 1], None, ALU.mult
            )

        # y[:, dh] = sum_e diag(w[:, e]) @ h_bf16[:, :, e]  (PSUM accumulate)
        for dh in range(2):
            hb = hbpool.tile([128, DH, NE], BF16, tag="hb")
            nc.vector.tensor_copy(hb[:], hts[(bt, dh)][:])
            yp = ypsum.tile([128, DH], F32, tag="yp")
            for e in range(NE):
                nc.tensor.matmul(
                    yp[:], diag[:, e, :], hb[:, :, e],
                    start=(e == 0), stop=(e == NE - 1),
                )
            ysb = ypool.tile([128, DH], F32, tag="ysb")
            nc.scalar.copy(ysb[:], yp[:])
            nc.gpsimd.dma_start(aps["y_out"][bs, dh * DH:(dh + 1) * DH], ysb[:])


_NC_CACHE = {}


def _get_nc():
    if "nc" not in _NC_CACHE:
        _NC_CACHE["nc"] = build_nc(B_FULL // NCORES, None, NCORES)
    return _NC_CACHE["nc"]


def kernel(h, x, sel_w, leaf_w, leaf_b):
    from concourse.bass_utils import run_bass_kernel_spmd

    nc = _get_nc()
    BL = B_FULL // NCORES
    selw2 = np.ascontiguousarray(np.asarray(sel_w, np.float32).reshape(NSEL, D_IN))
    leafw2 = np.ascontiguousarray(np.asarray(leaf_w, np.float32).reshape(NLEAF, D_IN))
    leafb2 = np.ascontiguousarray(np.asarray(leaf_b, np.float32).reshape(1, NLEAF))
    h = np.asarray(h, np.float32)
    x = np.asarray(x, np.float32)
    in_maps = []
    for i in range(NCORES):
        sl = slice(i * BL, (i + 1) * BL)
        in_maps.append({
            "h": np.ascontiguousarray(h[sl]),
            "x": np.ascontiguousarray(x[sl]),
            "sel_w": selw2,
            "leaf_w": leafw2,
            "leaf_b": leafb2,
        })
    res = run_bass_kernel_spmd(nc, in_maps, core_ids=list(range(NCORES)))
    y = np.concatenate([res.results[i]["y_out"] for i in range(NCORES)], 0)
    w = np.concatenate([res.results[i]["w_out"] for i in range(NCORES)], 0)
    ent = np.concatenate([res.results[i]["ent_out"] for i in range(NCORES)], 0)[:, 0]

    s = w < 1e-5
    soft = w.mean(0, dtype=np.float64).astype(np.float32)
    hard = (1.0 - s).mean(0, dtype=np.float64).astype(np.float32)
    s_concat = s.astype(np.float32)[:, None, None, :]
    reg = np.float32(0.01 * ent.mean(dtype=np.float64))
    return (y, soft, hard, s_concat, reg)
